# revision 10
# baseline (speedup 1.0000x reference)
"""BiMamba (bidirectional Mamba block + LN + FFN) Trainium2 Bass kernel.

Sharding (8 cores): 4 scan-sequences (fwd/bwd x batch, bwd fed host-flipped x)
x 2 halves of d_inner. Feature-on-partitions / time-on-free throughout.

Redesign vs baseline:
- Each core computes in_proj only for its own d_inner half (xc own + z own);
  the x_proj contraction over the full d_inner is completed with a pair
  AllReduce of the (128, L) x_proj partial sums.
- All large GEMMs run in bf16/fp16 (1 cycle/row + fast FWL weight loads).
- Scan phase all fp16: exp on Act engine, b/prod multiplies on DVE at the
  2x packed rate, tensor_tensor_scan fp16, state-sum via fp16 identity
  matmuls into PSUM.
- Direction merge + FFN input distribution via ONE ReduceScatter over quads
  that scatters along TIME: each core then owns a 256-column t-slice, does
  LN + the full FFN locally (weights streamed JIT), no further collectives.
  The bwd-core time flip is handled with per-core 0/1 flag columns scaling
  a straight and a reversed copy into separate RS slots (identical program
  on all cores).
"""
import sys, os, types, contextlib, ctypes

sys.path.insert(0, "/opt/trn_rl_repo")
import numpy as np

D_MODEL = 1024
D_STATE = 16
D_CONV = 4
D_INNER = 2048
DT_RANK = 64
L = 1024
HALF = D_INNER // 2          # 1024 d_inner per core
P = 128
NJ = HALF // P               # 8 d-blocks per core half
TCH = 512                    # matmul t-chunk
NT = L // TCH
KD = D_MODEL // P            # 8 k-chunks over d_model
NH1 = 4 * D_MODEL // P       # 32 ffn hidden blocks
LQ = L // 4                  # 256 t-slice per core after RS

_PAIRS = [[0, 1], [2, 3], [4, 5], [6, 7]]
_QUADS = [[0, 1, 4, 5], [2, 3, 6, 7]]
GP_N = ()


def _install_ntff_hook_shim(so_path="/opt/axon/libaxon_pjrt.so"):
    if "antenv.axon_hooks" in sys.modules:
        return
    try:
        lib = ctypes.CDLL(so_path)
    except OSError:
        return
    if not hasattr(lib, "axon_start_nrt_profile"):
        return
    lib.axon_start_nrt_profile.argtypes = [ctypes.POINTER(ctypes.c_int64), ctypes.c_size_t]
    lib.axon_start_nrt_profile.restype = ctypes.c_int64
    lib.axon_stop_nrt_profile.argtypes = [ctypes.c_char_p]
    lib.axon_stop_nrt_profile.restype = ctypes.c_int64

    @contextlib.contextmanager
    def _hook(output_dir, device_ids):
        import jax
        jax.devices()
        if device_ids:
            ids = (ctypes.c_int64 * len(device_ids))(*device_ids)
            rc = lib.axon_start_nrt_profile(ids, len(device_ids))
        else:
            rc = lib.axon_start_nrt_profile(None, 0)
        if rc != 0:
            raise RuntimeError(f"axon_start_nrt_profile rc={rc}")
        try:
            yield
        finally:
            n = lib.axon_stop_nrt_profile(str(output_dir).encode())
            print(f"profile: {n} file(s) written to {output_dir}", file=sys.stderr)

    mod = types.ModuleType("antenv.axon_hooks")
    mod.get_axon_ntff_profile_hook = lambda: _hook
    mod.set_axon_ntff_profile_hook = lambda h: None
    sys.modules["antenv.axon_hooks"] = mod


def _build_nc():
    from concourse import bacc, tile, mybir

    f32 = mybir.dt.float32
    bf16 = mybir.dt.bfloat16
    fp16 = mybir.dt.float16
    Alu = mybir.AluOpType
    Act = mybir.ActivationFunctionType

    nc = bacc.Bacc("TRN2", target_bir_lowering=False, debug=False, num_devices=8)

    def din(name, shape, dt):
        return nc.dram_tensor(name, list(shape), dt, kind="ExternalInput").ap()

    xT = din("xT", (D_MODEL, L), bf16)
    w_in = din("w_in", (16, P, KD, P), bf16)        # fb 0..7 xc-own, 8..15 z-own
    convw_cols = din("convw_cols", (P, NJ * D_CONV), f32)
    convb_cols = din("convb_cols", (P, NJ), f32)
    xpw = din("xpw", (P, NJ, P), fp16)              # [p, k, n] n: dt64|B16|C16|pad
    dtw = din("dtw", (DT_RANK, NJ, P), fp16)
    dtb_cols = din("dtb_cols", (P, NJ), f32)
    A_cols = din("A_cols", (P, NJ * D_STATE), f32)
    D_cols = din("D_cols", (P, NJ), f32)
    outw = din("outw", (NJ, P, NJ, P), fp16)        # [m, p(k-part), k, mp]
    lng_cols = din("lng_cols", (P, KD), f32)
    lnb_cols = din("lnb_cols", (P, KD), f32)
    w1m = din("w1m", (NH1, P, KD, P), bf16)
    b1_cols = din("b1_cols", (P, NH1), f32)
    w2m = din("w2m", (KD, P, NH1, P), bf16)
    b2_cols = din("b2_cols", (P, KD), f32)
    identh = din("identh", (P, P), fp16)
    ones_h = din("ones_h", (P, 2), fp16)            # col0: ones (stats lhsT)
    onesrow_h = din("onesrow_h", (1, P), fp16)      # bcast lhsT
    flags = din("flags", (P, 2), f32)               # col0 fwd, col1 bwd

    out_m = nc.dram_tensor("out_m", [D_MODEL, LQ], f32, kind="ExternalOutput").ap()

    with tile.TileContext(nc) as tc:
        with contextlib.ExitStack() as stk:
            cpool = stk.enter_context(tc.tile_pool(name="cpool", bufs=1))
            dram = stk.enter_context(tc.tile_pool(name="dram", bufs=1, space="DRAM"))

            def cload(src, shape, dt, tag):
                t = cpool.tile(list(shape), dt, tag=tag, name=tag)
                nc.sync.dma_start(t[:], src)
                return t

            # input activations first in the DMA queue
            xts = []
            for k in range(KD):
                xt_k = cpool.tile([P, L], bf16, tag=f"xt{k}", name=f"xt{k}")
                nc.sync.dma_start(xt_k[:], xT[k * P:(k + 1) * P, :])
                xts.append(xt_k)

            convw_sb = cload(convw_cols[:], (P, NJ * D_CONV), f32, "convw_sb")
            convb_sb = cload(convb_cols[:], (P, NJ), f32, "convb_sb")
            dtb_sb = cload(dtb_cols[:], (P, NJ), f32, "dtb_sb")
            A_sb = cload(A_cols[:], (P, NJ * D_STATE), f32, "A_sb")
            D_sb = cload(D_cols[:], (P, NJ), f32, "D_sb")
            lng_sb = cload(lng_cols[:], (P, KD), f32, "lng_sb")
            lnb_sb = cload(lnb_cols[:], (P, KD), f32, "lnb_sb")
            b1_sb = cload(b1_cols[:], (P, NH1), f32, "b1_sb")
            b2_sb = cload(b2_cols[:], (P, KD), f32, "b2_sb")
            ident_sb = cload(identh[:], (P, P), fp16, "ident_sb")
            ones_sb = cload(ones_h[:], (P, 2), fp16, "ones_sb")
            onesrow_sb = cload(onesrow_h[:], (1, P), fp16, "onesrow_sb")
            flags_sb = cload(flags[:], (P, 2), f32, "flags_sb")
            xpw_sb = cload(xpw[:], (P, NJ * P), fp16, "xpw_sb")
            dtw_sb = cload(dtw[:], (DT_RANK, NJ * P), fp16, "dtw_sb")

            dbl_in = dram.tile([P, L], fp16, name="dbl_in")
            dbl_out = dram.tile([P, L], fp16, name="dbl_out")
            bcB2 = dram.tile([D_STATE, 2 * L], fp16, name="bcB2")
            bcC2 = dram.tile([D_STATE, 2 * L], fp16, name="bcC2")
            arqs = [dram.tile([4, D_MODEL // 4, LQ], fp16, name=f"arq{i}")
                    for i in range(4)]
            rs_outs = [dram.tile([D_MODEL // 4, LQ], fp16, name=f"rso{i}")
                       for i in range(4)]

            # persistent SBUF (P1->P4/P5)
            sz_pool = stk.enter_context(tc.tile_pool(name="sz_pool", bufs=1))
            szs = [sz_pool.tile([P, L], fp16, tag=f"sz{j}", name=f"sz{j}")
                   for j in range(NJ)]
            dl_pool = stk.enter_context(tc.tile_pool(name="dl_pool", bufs=1))
            dpair = [dl_pool.tile([P, 2 * L], fp16, tag=f"dl{g}", name=f"dl{g}")
                     for g in range(NJ // 2)]
            deltas = [dpair[j // 2][:, (j % 2) * L:(j % 2 + 1) * L]
                      for j in range(NJ)]
            wv_pool = stk.enter_context(tc.tile_pool(name="wv_pool", bufs=1))
            wpair = [wv_pool.tile([P, 2 * L], fp16, tag=f"wv{g}", name=f"wv{g}")
                     for g in range(NJ // 2)]
            wvs = [wpair[j // 2][:, (j % 2) * L:(j % 2 + 1) * L]
                   for j in range(NJ)]
            g0_pool = stk.enter_context(tc.tile_pool(name="g0_pool", bufs=1))
            g0s = [g0_pool.tile([P, L], fp16, tag=f"g0{j}", name=f"g0{j}")
                   for j in range(NJ)]
            yg_pool = stk.enter_context(tc.tile_pool(name="yg_pool", bufs=1))
            ygs = [yg_pool.tile([P, L], fp16, tag=f"yg{j}", name=f"yg{j}")
                   for j in range(NJ)]

            # ================= P1-P3 =================
            with tc.tile_pool(name="xc_pool", bufs=1) as xc_pool, \
                 tc.tile_pool(name="p13", bufs=1) as p13, \
                 tc.tile_pool(name="psA", bufs=4, space="PSUM") as psA:
                xcpair = [xc_pool.tile([P, 2 * L], fp16, tag=f"xc{g}",
                                       name=f"xc{g}") for g in range(NJ // 2)]
                xcs = [xcpair[j // 2][:, (j % 2) * L:(j % 2 + 1) * L]
                       for j in range(NJ)]

                def in_proj_block(fb, tag):
                    lw = p13.tile([P, KD * P], bf16, tag=tag, name=f"{tag}_{fb}",
                                  bufs=2)
                    nc.sync.dma_start(lw[:], w_in[fb])
                    pss = []
                    for t in range(NT):
                        ps = psA.tile([P, TCH], f32, tag="ps", name=f"inp{fb}_{t}")
                        for k in range(KD):
                            nc.tensor.matmul(ps[:], lw[:, k * P:(k + 1) * P],
                                             xts[k][:, t * TCH:(t + 1) * TCH],
                                             start=(k == 0), stop=(k == KD - 1))
                        pss.append(ps)
                    return pss

                # P1a: xc own half + conv + silu
                for j in range(NJ):
                    xcp = p13.tile([P, L + D_CONV - 1], fp16, tag="xcp",
                                   name=f"xcp{j}", bufs=2)
                    nc.vector.memset(xcp[:, 0:D_CONV - 1], 0.0)
                    for t, ps in enumerate(in_proj_block(j, "lwx")):
                        nc.scalar.copy(
                            xcp[:, D_CONV - 1 + t * TCH:D_CONV - 1 + (t + 1) * TCH],
                            ps[:])
                    cacc = p13.tile([P, L], fp16, tag="cacc", name=f"cacc{j}",
                                    bufs=2)
                    nc.vector.tensor_scalar_mul(
                        cacc[:], xcp[:, 0:L],
                        convw_sb[:, j * D_CONV:j * D_CONV + 1])
                    for i in range(1, D_CONV):
                        nc.vector.scalar_tensor_tensor(
                            cacc[:], xcp[:, i:i + L],
                            convw_sb[:, j * D_CONV + i:j * D_CONV + i + 1],
                            cacc[:], Alu.mult, Alu.add)
                    nc.scalar.activation(xcs[j], cacc[:], Act.Silu,
                                         bias=convb_sb[:, j:j + 1])

                # P1b: x_proj partial over own half + pair AllReduce
                dblp = p13.tile([P, L], fp16, tag="dblp", name="dblp")
                for t in range(NT):
                    ps = psA.tile([P, TCH], f32, tag="ps", name=f"xp{t}")
                    for k in range(NJ):
                        nc.tensor.matmul(ps[:], xpw_sb[:, k * P:(k + 1) * P],
                                         xcs[k][:, t * TCH:(t + 1) * TCH],
                                         start=(k == 0), stop=(k == NJ - 1))
                    nc.scalar.copy(dblp[:, t * TCH:(t + 1) * TCH], ps[:])
                nc.sync.dma_start(dbl_in[:], dblp[:])
                nc.gpsimd.collective_compute(
                    "AllReduce", Alu.add, replica_groups=_PAIRS,
                    ins=[dbl_in[:]], outs=[dbl_out[:]])

                # P1c: z own half + silu, first half (overlaps the AllReduce)
                for j in range(NJ // 2):
                    for t, ps in enumerate(in_proj_block(NJ + j, "lwz")):
                        nc.scalar.activation(szs[j][:, t * TCH:(t + 1) * TCH],
                                             ps[:], Act.Silu)

                # P2: unpack AllReduce result (fp16 throughout)
                dt16 = p13.tile([DT_RANK, L], fp16, tag="dt16", name="dt16")
                nc.sync.dma_start(dt16[:], dbl_out[0:DT_RANK, :])
                nc.sync.dma_start(bcB2[:, 0:L],
                                  dbl_out[DT_RANK:DT_RANK + D_STATE, :])
                nc.sync.dma_start(bcB2[:, L:2 * L],
                                  dbl_out[DT_RANK:DT_RANK + D_STATE, :])
                nc.sync.dma_start(bcC2[:, 0:L],
                                  dbl_out[DT_RANK + D_STATE:DT_RANK + 2 * D_STATE, :])
                nc.sync.dma_start(bcC2[:, L:2 * L],
                                  dbl_out[DT_RANK + D_STATE:DT_RANK + 2 * D_STATE, :])

                # P2b: dt_proj + softplus -> delta (fp16); batch Exp then Ln
                spts = {}
                for j in range(NJ):
                    for t in range(NT):
                        ps = psA.tile([P, TCH], f32, tag="ps", name=f"dtp{j}_{t}")
                        nc.tensor.matmul(ps[:], dtw_sb[:, j * P:(j + 1) * P],
                                         dt16[:, t * TCH:(t + 1) * TCH],
                                         start=True, stop=True)
                        spt = p13.tile([P, TCH], fp16, tag=f"sp{j}_{t}",
                                       name=f"spt{j}_{t}")
                        nc.scalar.activation(spt[:], ps[:], Act.Exp,
                                             bias=dtb_sb[:, j:j + 1])
                        spts[(j, t)] = spt
                for j in range(NJ):
                    for t in range(NT):
                        nc.scalar.activation(deltas[j][:, t * TCH:(t + 1) * TCH],
                                             spts[(j, t)][:], Act.Ln, bias=1.0)

                # P1c cont: z own half, second half (after dt_proj matmuls)
                for j in range(NJ // 2, NJ):
                    for t, ps in enumerate(in_proj_block(NJ + j, "lwz")):
                        nc.scalar.activation(szs[j][:, t * TCH:(t + 1) * TCH],
                                             ps[:], Act.Silu)

                # P3: wv (paired), g0 (per j), then clobber pair-boundary delta
                for g in range(NJ // 2):
                    nc.vector.tensor_tensor(wpair[g][:], dpair[g][:],
                                            xcpair[g][:], Alu.mult)
                    nc.vector.memset(dpair[g][:, L:L + 1], 30.0)
                for j in range(NJ):
                    t1 = p13.tile([P, L], fp16, tag="g0t", name=f"g0t{j}", bufs=2)
                    nc.vector.tensor_scalar_mul(t1[:], xcs[j], D_sb[:, j:j + 1])
                    nc.vector.tensor_tensor(g0s[j][:], t1[:], szs[j][:], Alu.mult)

            # out_proj weights (2 MB fp16) load during the scan phase
            outw_sb = [cload(outw[m], (P, NJ * P), fp16, f"outw{m}")
                       for m in range(NJ)]

            # ================= P4: scan =================
            with tc.tile_pool(name="p4t", bufs=1) as p4t, \
                 tc.tile_pool(name="pscan", bufs=1, space="PSUM") as pscan:
                for hb in range(4):
                    js = [hb * 2, hb * 2 + 1]
                    yps = {j: pscan.tile([P, L], f32, tag=f"yps{hb % 2}_{j % 2}",
                                         name=f"yps{j}") for j in js}
                    for n in range(D_STATE):
                        bcb = p4t.tile([P, 2 * L], fp16, tag="bcb",
                                       name=f"bcb{hb}_{n}", bufs=3)
                        nc.sync.dma_start(
                            bcb[:],
                            bcB2[n:n + 1, :].partition_broadcast(P).squeeze(1))
                        bcc = p4t.tile([P, 2 * L], fp16, tag="bcc",
                                       name=f"bcc{hb}_{n}", bufs=3)
                        nc.sync.dma_start(
                            bcc[:],
                            bcC2[n:n + 1, :].partition_broadcast(P).squeeze(1))
                        a_p = p4t.tile([P, 2 * L], fp16, tag="a_p",
                                       name=f"a{hb}_{n}", bufs=3)
                        for j in js:
                            nc.scalar.activation(
                                a_p[:, (j % 2) * L:(j % 2 + 1) * L],
                                deltas[j], Act.Exp,
                                scale=A_sb[:, j * D_STATE + n:j * D_STATE + n + 1])
                        b_p = p4t.tile([P, 2 * L], fp16, tag="b_p",
                                       name=f"b{hb}_{n}", bufs=2)
                        nc.vector.tensor_tensor(b_p[:], wpair[hb][:], bcb[:],
                                                Alu.mult)
                        h_p = p4t.tile([P, 2 * L], fp16, tag="h_p",
                                       name=f"h{hb}_{n}", bufs=2)
                        nc.vector.tensor_tensor_scan(h_p[:], a_p[:], b_p[:],
                                                     0.0, Alu.mult, Alu.add)
                        prod = p4t.tile([P, 2 * L], fp16, tag="prod",
                                        name=f"p{hb}_{n}", bufs=4)
                        nc.vector.tensor_tensor(prod[:], h_p[:], bcc[:], Alu.mult)
                        for j in js:
                            for t in range(NT):
                                sl = slice((j % 2) * L + t * TCH,
                                           (j % 2) * L + (t + 1) * TCH)
                                nc.tensor.matmul(
                                    yps[j][:, t * TCH:(t + 1) * TCH],
                                    ident_sb[:], prod[:, sl],
                                    start=(n == 0), stop=(n == D_STATE - 1))
                    for j in js:
                        yc = p4t.tile([P, L], fp16, tag="yc", name=f"yc{j}",
                                      bufs=2)
                        nc.scalar.copy(yc[:], yps[j][:])
                        ygt = p4t.tile([P, L], fp16, tag="ygt", name=f"ygt{j}",
                                       bufs=2)
                        nc.vector.tensor_tensor(ygt[:], yc[:], szs[j][:], Alu.mult)
                        nc.vector.tensor_tensor(ygs[j][:], ygt[:], g0s[j][:],
                                                Alu.add)

            # ================= P5: out_proj + RS =================
            with tc.tile_pool(name="p5t", bufs=1) as p5t, \
                 tc.tile_pool(name="psC", bufs=1, space="PSUM") as psC:
                for m in range(NJ):
                    ms = p5t.tile([P, L], fp16, tag="ms", name=f"ms{m}", bufs=2)
                    for t in range(NT):
                        ps = psC.tile([P, TCH], f32, tag="ps", name=f"op{m}_{t}", bufs=2)
                        for k in range(NJ):
                            nc.tensor.matmul(ps[:],
                                             outw_sb[m][:, k * P:(k + 1) * P],
                                             ygs[k][:, t * TCH:(t + 1) * TCH],
                                             start=(k == 0), stop=(k == NJ - 1))
                        nc.scalar.copy(ms[:, t * TCH:(t + 1) * TCH], ps[:])
                    msF = p5t.tile([P, L], fp16, tag="msF", name=f"msF{m}", bufs=2)
                    nc.scalar.activation(msF[:], ms[:], Act.Copy,
                                         scale=flags_sb[:, 0:1])
                    msB = p5t.tile([P, L], fp16, tag="msB", name=f"msB{m}", bufs=2)
                    nc.scalar.activation(msB[:], ms[:, ::-1], Act.Copy,
                                         scale=flags_sb[:, 1:2])
                    msb = p5t.tile([P, L], fp16, tag="msb", name=f"msb{m}", bufs=2)
                    nc.vector.tensor_tensor(msb[:], msF[:], msB[:], Alu.add)
                    dst = arqs[m // 2]
                    for q in range(4):
                        nc.sync.dma_start(
                            dst[q, (m % 2) * P:(m % 2 + 1) * P, :],
                            msb[:, q * LQ:(q + 1) * LQ])
                    if m % 2 == 1:
                        nc.gpsimd.collective_compute(
                            "ReduceScatter", Alu.add, replica_groups=_QUADS,
                            ins=[arqs[m // 2][:]], outs=[rs_outs[m // 2][:]])

                # ================= P6: merge + LN =================
                # prefetch first FFN weight tiles while the RS is in flight
                w1_pre = []
                for m in range(6):
                    lw = p5t.tile([P, KD * P], bf16, tag="w1", name=f"w1_{m}",
                                  bufs=6)
                    nc.sync.dma_start(lw[:], w1m[m])
                    w1_pre.append(lw)
                w2_pre = p5t.tile([P, NH1 * P], bf16, tag="w2", name="w2_0",
                                  bufs=2)
                nc.sync.dma_start(w2_pre[:], w2m[0])

                mos = [p5t.tile([P, LQ], fp16, tag=f"mo{j}", name=f"mo{j}")
                       for j in range(KD)]
                mu_ps = psC.tile([1, LQ], f32, tag="mu", name="mu_ps")
                e2_ps = psC.tile([1, LQ], f32, tag="e2", name="e2_ps")
                for j in range(KD):
                    nc.sync.dma_start(mos[j][:],
                                      rs_outs[j // 2][(j % 2) * P:(j % 2 + 1) * P, :])
                    sq = p5t.tile([P, LQ], fp16, tag="sq", name=f"sq{j}", bufs=2)
                    nc.scalar.activation(sq[:], mos[j][:], Act.Square)
                    nc.tensor.matmul(mu_ps[:], ones_sb[:, 0:1], mos[j][:],
                                     start=(j == 0), stop=(j == KD - 1))
                    nc.tensor.matmul(e2_ps[:], ones_sb[:, 0:1], sq[:],
                                     start=(j == 0), stop=(j == KD - 1))
                mean = p5t.tile([1, LQ], f32, tag="mean", name="mean")
                nc.scalar.activation(mean[:], mu_ps[:], Act.Copy,
                                     scale=1.0 / D_MODEL)
                e2m = p5t.tile([1, LQ], f32, tag="e2m", name="e2m")
                nc.scalar.activation(e2m[:], e2_ps[:], Act.Copy,
                                     scale=1.0 / D_MODEL)
                m2 = p5t.tile([1, LQ], f32, tag="m2", name="m2")
                nc.vector.tensor_tensor(m2[:], mean[:], mean[:], Alu.mult)
                var = p5t.tile([1, LQ], f32, tag="var", name="var")
                nc.vector.tensor_tensor(var[:], e2m[:], m2[:], Alu.subtract)
                eps_sb = p5t.tile([1, 1], f32, tag="eps", name="eps_sb")
                nc.vector.memset(eps_sb[:], 1e-5)
                std = p5t.tile([1, LQ], f32, tag="std", name="std")
                nc.scalar.activation(std[:], var[:], Act.Sqrt, bias=eps_sb[:])
                rstd = p5t.tile([1, LQ], f32, tag="rstd", name="rstd")
                nc.vector.reciprocal(rstd[:], std[:])
                mean_h = p5t.tile([1, LQ], fp16, tag="mean_h", name="mean_h")
                nc.scalar.copy(mean_h[:], mean[:])
                rstd_h = p5t.tile([1, LQ], fp16, tag="rstd_h", name="rstd_h")
                nc.scalar.copy(rstd_h[:], rstd[:])
                mean_bc = psC.tile([P, LQ], f32, tag="mbc", name="mean_bc")
                nc.tensor.matmul(mean_bc[:], onesrow_sb[:], mean_h[:],
                                 start=True, stop=True)
                rstd_bc = psC.tile([P, LQ], f32, tag="rbc", name="rstd_bc")
                nc.tensor.matmul(rstd_bc[:], onesrow_sb[:], rstd_h[:],
                                 start=True, stop=True)

                xns = [p5t.tile([P, LQ], bf16, tag=f"xn{j}", name=f"xn{j}")
                       for j in range(KD)]
                for j in range(KD):
                    t1 = p5t.tile([P, LQ], f32, tag="lnt", name=f"lnt{j}", bufs=2)
                    nc.vector.tensor_tensor(t1[:], mos[j][:], mean_bc[:],
                                            Alu.subtract)
                    nc.vector.tensor_tensor(t1[:], t1[:], rstd_bc[:], Alu.mult)
                    nc.vector.tensor_scalar(xns[j][:], t1[:], lng_sb[:, j:j + 1],
                                            lnb_sb[:, j:j + 1], Alu.mult, Alu.add)

                # ================= P7: FFN =================
                with tc.tile_pool(name="ffh_pool", bufs=1) as ffh_pool:
                    ffhs = [ffh_pool.tile([P, LQ], bf16, tag=f"fh{m}",
                                          name=f"fh{m}") for m in range(NH1)]
                    for m in range(NH1):
                        if m < 6:
                            lw = w1_pre[m]
                        else:
                            lw = p5t.tile([P, KD * P], bf16, tag="w1",
                                          name=f"w1_{m}", bufs=6)
                            nc.sync.dma_start(lw[:], w1m[m])
                        ps = psC.tile([P, LQ], f32, tag="psf", name=f"f1{m}",
                                      bufs=2)
                        for k in range(KD):
                            nc.tensor.matmul(ps[:], lw[:, k * P:(k + 1) * P],
                                             xns[k][:],
                                             start=(k == 0), stop=(k == KD - 1))
                        nc.scalar.activation(ffhs[m][:], ps[:], Act.Gelu,
                                             bias=b1_sb[:, m:m + 1])

                    for m in range(KD):
                        if m == 0:
                            lw = w2_pre
                        else:
                            lw = p5t.tile([P, NH1 * P], bf16, tag="w2",
                                          name=f"w2_{m}", bufs=2)
                            nc.sync.dma_start(lw[:], w2m[m])
                        ps = psC.tile([P, LQ], f32, tag="psf", name=f"f2{m}",
                                      bufs=2)
                        for k in range(NH1):
                            nc.tensor.matmul(ps[:], lw[:, k * P:(k + 1) * P],
                                             ffhs[k][:],
                                             start=(k == 0), stop=(k == NH1 - 1))
                        ob = p5t.tile([P, LQ], f32, tag="ob", name=f"ob{m}",
                                      bufs=2)
                        nc.vector.tensor_scalar_add(ob[:], ps[:],
                                                    b2_sb[:, m:m + 1])
                        nc.sync.dma_start(out_m[m * P:(m + 1) * P, :], ob[:])

    nc.compile()
    return nc


def _prep_inputs(inputs):
    """Per-core input dicts. Core c: sequence s=c//2 (s>=2 => time-flipped x),
    d_inner half = c%2."""
    import ml_dtypes
    bf = ml_dtypes.bfloat16
    fh = np.float16

    x = np.asarray(inputs["x"], dtype=np.float32)
    in_proj_w = np.asarray(inputs["in_proj_w"], dtype=np.float32)
    conv_w = np.asarray(inputs["conv_w"], dtype=np.float32)
    conv_b = np.asarray(inputs["conv_b"], dtype=np.float32)
    x_proj_w = np.asarray(inputs["x_proj_w"], dtype=np.float32)
    dt_proj_w = np.asarray(inputs["dt_proj_w"], dtype=np.float32)
    dt_proj_b = np.asarray(inputs["dt_proj_b"], dtype=np.float32)
    A = -np.exp(np.asarray(inputs["A_log"], dtype=np.float32))
    Dp = np.asarray(inputs["D"], dtype=np.float32)
    out_proj_w = np.asarray(inputs["out_proj_w"], dtype=np.float32)
    ln_g = np.asarray(inputs["ln_g"], dtype=np.float32)
    ln_b = np.asarray(inputs["ln_b"], dtype=np.float32)
    ff_w1 = np.asarray(inputs["ff_w1"], dtype=np.float32)
    ff_b1 = np.asarray(inputs["ff_b1"], dtype=np.float32)
    ff_w2 = np.asarray(inputs["ff_w2"], dtype=np.float32)
    ff_b2 = np.asarray(inputs["ff_b2"], dtype=np.float32)

    def cols(v):  # (N,) -> (P, N//P)
        return np.ascontiguousarray(v.reshape(-1, P).T)

    def wblocks(w, dt):  # (K, M) -> (M//P, P(kpart), K//P, P(m))
        K, M = w.shape
        r = w.reshape(K // P, P, M // P, P).transpose(2, 1, 0, 3)
        return np.ascontiguousarray(r.astype(dt))

    # shared across cores
    w1_t = wblocks(ff_w1, bf)                      # (32, P, 8, P)
    w2_t = wblocks(ff_w2, bf)                      # (8, P, 32, P)
    lngc = cols(ln_g)
    lnbc = cols(ln_b)
    b1c = cols(ff_b1)
    b2c = cols(ff_b2)
    identh = np.eye(P, dtype=fh)
    ones_h = np.ones((P, 2), fh)
    onesrow_h = np.ones((1, P), fh)

    in_maps = []
    for c in range(8):
        s, half = c // 2, c % 2
        xb = x[s] if s < 2 else x[s - 2][::-1]
        own = np.arange(half * HALF, (half + 1) * HALF)

        wxc = in_proj_w[:, own]                    # (1024, 1024)
        wz = in_proj_w[:, D_INNER + own]
        w_in = np.concatenate(
            [wblocks(wxc, bf), wblocks(wz, bf)], axis=0)   # (16, P, 8, P)

        cw = conv_w[own]
        convw_cols = np.ascontiguousarray(
            cw.reshape(NJ, P, D_CONV).transpose(1, 0, 2).reshape(P, NJ * D_CONV))

        xp = np.concatenate(
            [x_proj_w[own], np.zeros((HALF, P - DT_RANK - 2 * D_STATE),
                                     np.float32)], axis=1)  # (1024, 128)
        xpw_t = wblocks(xp, fh)[0]                 # (P, 8, P)

        dtw_t = np.ascontiguousarray(
            dt_proj_w[:, own].reshape(DT_RANK, NJ, P).astype(fh))

        A_colsv = np.ascontiguousarray(
            A[own].reshape(NJ, P, D_STATE).transpose(1, 0, 2).reshape(
                P, NJ * D_STATE))

        outw_t = wblocks(out_proj_w[own], fh)      # (8, P, 8, P)

        fwd = 1.0 if s < 2 else 0.0
        flags = np.concatenate([np.full((P, 1), fwd, np.float32),
                                np.full((P, 1), 1.0 - fwd, np.float32)], axis=1)

        in_maps.append({
            "xT": np.ascontiguousarray(xb.T).astype(bf),
            "w_in": w_in,
            "convw_cols": convw_cols,
            "convb_cols": cols(conv_b[own]),
            "xpw": xpw_t,
            "dtw": dtw_t,
            "dtb_cols": cols(dt_proj_b[own]),
            "A_cols": A_colsv,
            "D_cols": cols(Dp[own]),
            "outw": outw_t,
            "lng_cols": lngc,
            "lnb_cols": lnbc,
            "w1m": w1_t,
            "b1_cols": b1c,
            "w2m": w2_t,
            "b2_cols": b2c,
            "identh": identh,
            "ones_h": ones_h,
            "onesrow_h": onesrow_h,
            "flags": flags,
        })
    return in_maps


_NC_CACHE = {}


def _get_nc():
    if "nc" not in _NC_CACHE:
        _NC_CACHE["nc"] = _build_nc()
    return _NC_CACHE["nc"]


def run(inputs, trace=False):
    _install_ntff_hook_shim()
    from concourse import bass_utils
    nc = _get_nc()
    in_maps = _prep_inputs(inputs)
    res = bass_utils.run_bass_kernel_spmd(nc, in_maps, core_ids=list(range(8)),
                                          trace=trace)
    # core at quad-rank q holds t-columns [q*256, (q+1)*256) of its batch
    full = np.zeros((2, D_MODEL, L), np.float32)
    for c in range(8):
        b = 0 if c in _QUADS[0] else 1
        q = _QUADS[b].index(c)
        full[b, :, q * LQ:(q + 1) * LQ] = res.results[c]["out_m"]
    out = np.ascontiguousarray(full.transpose(0, 2, 1))
    return out, res


def kernel(**inputs):
    out, _ = run(inputs, trace=False)
    return out


# revision 16
# speedup vs baseline: 1.0527x; 1.0527x over previous
"""BiMamba (bidirectional Mamba block + LN + FFN) Trainium2 Bass kernel.

Sharding (8 cores): 4 scan-sequences (fwd/bwd x batch, bwd fed host-flipped x)
x 2 halves of d_inner. Feature-on-partitions / time-on-free throughout.

Redesign vs baseline:
- Each core computes in_proj only for its own d_inner half (xc own + z own);
  the x_proj contraction over the full d_inner is completed with a pair
  AllReduce of the (128, L) x_proj partial sums.
- All large GEMMs run in bf16/fp16 (1 cycle/row + fast FWL weight loads).
- Scan phase all fp16: exp on Act engine, b/prod multiplies on DVE at the
  2x packed rate, tensor_tensor_scan fp16, state-sum via fp16 identity
  matmuls into PSUM.
- Direction merge + FFN input distribution via ONE ReduceScatter over quads
  that scatters along TIME: each core then owns a 256-column t-slice, does
  LN + the full FFN locally (weights streamed JIT), no further collectives.
  The bwd-core time flip is handled with per-core 0/1 flag columns scaling
  a straight and a reversed copy into separate RS slots (identical program
  on all cores).
"""
import sys, os, types, contextlib, ctypes

sys.path.insert(0, "/opt/trn_rl_repo")
import numpy as np

D_MODEL = 1024
D_STATE = 16
D_CONV = 4
D_INNER = 2048
DT_RANK = 64
L = 1024
HALF = D_INNER // 2          # 1024 d_inner per core
P = 128
NJ = HALF // P               # 8 d-blocks per core half
TCH = 512                    # matmul t-chunk
NT = L // TCH
KD = D_MODEL // P            # 8 k-chunks over d_model
NH1 = 4 * D_MODEL // P       # 32 ffn hidden blocks
LQ = L // 4                  # 256 t-slice per core after RS

_PAIRS = [[0, 1], [2, 3], [4, 5], [6, 7]]
_QUADS = [[0, 1, 4, 5], [2, 3, 6, 7]]
GP_N = ()


def _install_ntff_hook_shim(so_path="/opt/axon/libaxon_pjrt.so"):
    if "antenv.axon_hooks" in sys.modules:
        return
    try:
        lib = ctypes.CDLL(so_path)
    except OSError:
        return
    if not hasattr(lib, "axon_start_nrt_profile"):
        return
    lib.axon_start_nrt_profile.argtypes = [ctypes.POINTER(ctypes.c_int64), ctypes.c_size_t]
    lib.axon_start_nrt_profile.restype = ctypes.c_int64
    lib.axon_stop_nrt_profile.argtypes = [ctypes.c_char_p]
    lib.axon_stop_nrt_profile.restype = ctypes.c_int64

    @contextlib.contextmanager
    def _hook(output_dir, device_ids):
        import jax
        jax.devices()
        if device_ids:
            ids = (ctypes.c_int64 * len(device_ids))(*device_ids)
            rc = lib.axon_start_nrt_profile(ids, len(device_ids))
        else:
            rc = lib.axon_start_nrt_profile(None, 0)
        if rc != 0:
            raise RuntimeError(f"axon_start_nrt_profile rc={rc}")
        try:
            yield
        finally:
            n = lib.axon_stop_nrt_profile(str(output_dir).encode())
            print(f"profile: {n} file(s) written to {output_dir}", file=sys.stderr)

    mod = types.ModuleType("antenv.axon_hooks")
    mod.get_axon_ntff_profile_hook = lambda: _hook
    mod.set_axon_ntff_profile_hook = lambda h: None
    sys.modules["antenv.axon_hooks"] = mod


def _build_nc():
    from concourse import bacc, tile, mybir

    f32 = mybir.dt.float32
    bf16 = mybir.dt.bfloat16
    fp16 = mybir.dt.float16
    Alu = mybir.AluOpType
    Act = mybir.ActivationFunctionType

    nc = bacc.Bacc("TRN2", target_bir_lowering=False, debug=False, num_devices=8)

    def din(name, shape, dt):
        return nc.dram_tensor(name, list(shape), dt, kind="ExternalInput").ap()

    xT = din("xT", (D_MODEL, L), bf16)
    w_in = din("w_in", (16, P, KD, P), bf16)        # fb 0..7 xc-own, 8..15 z-own
    convw_cols = din("convw_cols", (P, NJ * D_CONV), f32)
    convb_cols = din("convb_cols", (P, NJ), f32)
    xpw = din("xpw", (P, NJ, P), fp16)              # [p, k, n] n: dt64|B16|C16|pad
    dtw = din("dtw", (DT_RANK, NJ, P), fp16)
    dtb_cols = din("dtb_cols", (P, NJ), f32)
    A_cols = din("A_cols", (P, NJ * D_STATE), f32)
    D_cols = din("D_cols", (P, NJ), f32)
    outw = din("outw", (NJ, P, NJ, P), fp16)        # [m, p(k-part), k, mp]
    lng_cols = din("lng_cols", (P, KD), f32)
    lnb_cols = din("lnb_cols", (P, KD), f32)
    w1m = din("w1m", (NH1, P, KD, P), bf16)
    b1_cols = din("b1_cols", (P, NH1), f32)
    w2m = din("w2m", (KD, P, NH1, P), bf16)
    b2_cols = din("b2_cols", (P, KD), f32)
    identh = din("identh", (P, P), fp16)
    Ddiag = din("Ddiag", (P, NJ, P), fp16)
    ones_h = din("ones_h", (P, 2), fp16)            # col0: ones (stats lhsT)
    onesrow_h = din("onesrow_h", (1, P), fp16)      # bcast lhsT
    flags = din("flags", (P, 2), f32)               # col0 fwd, col1 bwd

    out_m = nc.dram_tensor("out_m", [D_MODEL, LQ], f32, kind="ExternalOutput").ap()

    with tile.TileContext(nc) as tc:
        with contextlib.ExitStack() as stk:
            cpool = stk.enter_context(tc.tile_pool(name="cpool", bufs=1))
            dram = stk.enter_context(tc.tile_pool(name="dram", bufs=1, space="DRAM"))

            def cload(src, shape, dt, tag):
                t = cpool.tile(list(shape), dt, tag=tag, name=tag)
                nc.sync.dma_start(t[:], src)
                return t

            # input activations first in the DMA queue
            xts = []
            for k in range(KD):
                xt_k = cpool.tile([P, L], bf16, tag=f"xt{k}", name=f"xt{k}")
                nc.sync.dma_start(xt_k[:], xT[k * P:(k + 1) * P, :])
                xts.append(xt_k)

            convw_sb = cload(convw_cols[:], (P, NJ * D_CONV), f32, "convw_sb")
            convb_sb = cload(convb_cols[:], (P, NJ), f32, "convb_sb")
            dtb_sb = cload(dtb_cols[:], (P, NJ), f32, "dtb_sb")
            A_sb = cload(A_cols[:], (P, NJ * D_STATE), f32, "A_sb")
            D_sb = cload(D_cols[:], (P, NJ), f32, "D_sb")
            lng_sb = cload(lng_cols[:], (P, KD), f32, "lng_sb")
            lnb_sb = cload(lnb_cols[:], (P, KD), f32, "lnb_sb")
            b1_sb = cload(b1_cols[:], (P, NH1), f32, "b1_sb")
            b2_sb = cload(b2_cols[:], (P, KD), f32, "b2_sb")
            ident_sb = cload(identh[:], (P, P), fp16, "ident_sb")
            Ddiag_sb = cload(Ddiag[:], (P, NJ * P), fp16, "Ddiag_sb")
            ones_sb = cload(ones_h[:], (P, 2), fp16, "ones_sb")
            onesrow_sb = cload(onesrow_h[:], (1, P), fp16, "onesrow_sb")
            flags_sb = cload(flags[:], (P, 2), f32, "flags_sb")
            xpw_sb = cload(xpw[:], (P, NJ * P), fp16, "xpw_sb")
            dtw_sb = cload(dtw[:], (DT_RANK, NJ * P), fp16, "dtw_sb")

            warm_in = dram.tile([P, 2], fp16, name="warm_in")
            warm_out = dram.tile([P, 2], fp16, name="warm_out")
            wtile = cpool.tile([P, 2], fp16, tag="wtile", name="wtile")
            nc.vector.memset(wtile[:], 0.0)
            nc.sync.dma_start(warm_in[:], wtile[:])
            nc.gpsimd.collective_compute(
                "AllReduce", Alu.add, replica_groups=_PAIRS,
                ins=[warm_in[:]], outs=[warm_out[:]])

            dbl_in = dram.tile([P, L], fp16, name="dbl_in")
            dbl_out = dram.tile([P, L], fp16, name="dbl_out")
            bcB2 = dram.tile([D_STATE, 2 * L], fp16, name="bcB2")
            bcC2 = dram.tile([D_STATE, 2 * L], fp16, name="bcC2")
            arqs = [dram.tile([4, D_MODEL // 2, LQ], fp16, name=f"arq{i}")
                    for i in range(2)]
            rs_outs = [dram.tile([D_MODEL // 2, LQ], fp16, name=f"rso{i}")
                       for i in range(2)]

            # persistent SBUF (P1->P4/P5)
            sz_pool = stk.enter_context(tc.tile_pool(name="sz_pool", bufs=1))
            szs = [sz_pool.tile([P, L], fp16, tag=f"sz{j}", name=f"sz{j}")
                   for j in range(NJ)]
            dl_pool = stk.enter_context(tc.tile_pool(name="dl_pool", bufs=1))
            dpair = [dl_pool.tile([P, 2 * L], fp16, tag=f"dl{g}", name=f"dl{g}")
                     for g in range(NJ // 2)]
            deltas = [dpair[j // 2][:, (j % 2) * L:(j % 2 + 1) * L]
                      for j in range(NJ)]
            wv_pool = stk.enter_context(tc.tile_pool(name="wv_pool", bufs=1))
            wpair = [wv_pool.tile([P, 2 * L], fp16, tag=f"wv{g}", name=f"wv{g}")
                     for g in range(NJ // 2)]
            wvs = [wpair[j // 2][:, (j % 2) * L:(j % 2 + 1) * L]
                   for j in range(NJ)]
            xc_pool = stk.enter_context(tc.tile_pool(name="xc_pool", bufs=1))
            xcpair = [xc_pool.tile([P, 2 * L], fp16, tag=f"xc{g}",
                                   name=f"xc{g}") for g in range(NJ // 2)]
            xcs = [xcpair[j // 2][:, (j % 2) * L:(j % 2 + 1) * L]
                   for j in range(NJ)]
            yg_pool = stk.enter_context(tc.tile_pool(name="yg_pool", bufs=1))
            ygs = [yg_pool.tile([P, L], fp16, tag=f"yg{j}", name=f"yg{j}")
                   for j in range(NJ)]

            # ================= P1-P3 =================
            with tc.tile_pool(name="p13", bufs=1) as p13, \
                 tc.tile_pool(name="psA", bufs=4, space="PSUM") as psA:

                def in_proj_block(fb, tag):
                    lw = p13.tile([P, KD * P], bf16, tag=tag, name=f"{tag}_{fb}",
                                  bufs=2)
                    nc.sync.dma_start(lw[:], w_in[fb])
                    pss = []
                    for t in range(NT):
                        ps = psA.tile([P, TCH], f32, tag="ps", name=f"inp{fb}_{t}")
                        for k in range(KD):
                            nc.tensor.matmul(ps[:], lw[:, k * P:(k + 1) * P],
                                             xts[k][:, t * TCH:(t + 1) * TCH],
                                             start=(k == 0), stop=(k == KD - 1))
                        pss.append(ps)
                    return pss

                # P1a: xc own half + conv + silu
                for j in range(NJ):
                    xcp = p13.tile([P, L + D_CONV - 1], fp16, tag="xcp",
                                   name=f"xcp{j}", bufs=2)
                    nc.vector.memset(xcp[:, 0:D_CONV - 1], 0.0)
                    for t, ps in enumerate(in_proj_block(j, "lwx")):
                        nc.scalar.copy(
                            xcp[:, D_CONV - 1 + t * TCH:D_CONV - 1 + (t + 1) * TCH],
                            ps[:])
                    cacc = p13.tile([P, L], fp16, tag="cacc", name=f"cacc{j}",
                                    bufs=2)
                    nc.vector.tensor_scalar_mul(
                        cacc[:], xcp[:, 0:L],
                        convw_sb[:, j * D_CONV:j * D_CONV + 1])
                    for i in range(1, D_CONV):
                        nc.vector.scalar_tensor_tensor(
                            cacc[:], xcp[:, i:i + L],
                            convw_sb[:, j * D_CONV + i:j * D_CONV + i + 1],
                            cacc[:], Alu.mult, Alu.add)
                    nc.scalar.activation(xcs[j], cacc[:], Act.Silu,
                                         bias=convb_sb[:, j:j + 1])

                # P1b: x_proj partial over own half + pair AllReduce
                dblp = p13.tile([P, L], fp16, tag="dblp", name="dblp")
                for t in range(NT):
                    ps = psA.tile([P, TCH], f32, tag="ps", name=f"xp{t}")
                    for k in range(NJ):
                        nc.tensor.matmul(ps[:], xpw_sb[:, k * P:(k + 1) * P],
                                         xcs[k][:, t * TCH:(t + 1) * TCH],
                                         start=(k == 0), stop=(k == NJ - 1))
                    nc.scalar.copy(dblp[:, t * TCH:(t + 1) * TCH], ps[:])
                nc.sync.dma_start(dbl_in[:], dblp[:])
                nc.gpsimd.collective_compute(
                    "AllReduce", Alu.add, replica_groups=_PAIRS,
                    ins=[dbl_in[:]], outs=[dbl_out[:]])

                # P1c: z own half + silu, first half (overlaps the AllReduce)
                for j in range(NJ // 2):
                    for t, ps in enumerate(in_proj_block(NJ + j, "lwz")):
                        nc.scalar.activation(szs[j][:, t * TCH:(t + 1) * TCH],
                                             ps[:], Act.Silu)

                # P2: unpack AllReduce result (fp16 throughout)
                dt16 = p13.tile([DT_RANK, L], fp16, tag="dt16", name="dt16")
                nc.sync.dma_start(dt16[:], dbl_out[0:DT_RANK, :])
                nc.sync.dma_start(bcB2[:, 0:L],
                                  dbl_out[DT_RANK:DT_RANK + D_STATE, :])
                nc.sync.dma_start(bcB2[:, L:2 * L],
                                  dbl_out[DT_RANK:DT_RANK + D_STATE, :])
                nc.sync.dma_start(bcC2[:, 0:L],
                                  dbl_out[DT_RANK + D_STATE:DT_RANK + 2 * D_STATE, :])
                nc.sync.dma_start(bcC2[:, L:2 * L],
                                  dbl_out[DT_RANK + D_STATE:DT_RANK + 2 * D_STATE, :])

                # P2b: dt_proj + softplus -> delta (fp16); batch Exp then Ln
                spts = {}
                for j in range(NJ):
                    for t in range(NT):
                        ps = psA.tile([P, TCH], f32, tag="ps", name=f"dtp{j}_{t}")
                        nc.tensor.matmul(ps[:], dtw_sb[:, j * P:(j + 1) * P],
                                         dt16[:, t * TCH:(t + 1) * TCH],
                                         start=True, stop=True)
                        spt = p13.tile([P, TCH], fp16, tag=f"sp{j}_{t}",
                                       name=f"spt{j}_{t}")
                        nc.scalar.activation(spt[:], ps[:], Act.Exp,
                                             bias=dtb_sb[:, j:j + 1])
                        spts[(j, t)] = spt
                for j in range(NJ):
                    for t in range(NT):
                        nc.scalar.activation(deltas[j][:, t * TCH:(t + 1) * TCH],
                                             spts[(j, t)][:], Act.Ln, bias=1.0)

                # P1c cont: z own half, second half (after dt_proj matmuls)
                for j in range(NJ // 2, NJ):
                    for t, ps in enumerate(in_proj_block(NJ + j, "lwz")):
                        nc.scalar.activation(szs[j][:, t * TCH:(t + 1) * TCH],
                                             ps[:], Act.Silu)

                # P3: wv (paired), then clobber pair-boundary delta
                for g in range(NJ // 2):
                    nc.vector.tensor_tensor(wpair[g][:], dpair[g][:],
                                            xcpair[g][:], Alu.mult)
                    nc.vector.memset(dpair[g][:, L:L + 1], 30.0)

            # out_proj weights (2 MB fp16) load during the scan phase
            outw_sb = [cload(outw[m], (P, NJ * P), fp16, f"outw{m}")
                       for m in range(NJ)]

            # ================= P4: scan =================
            with tc.tile_pool(name="p4t", bufs=1) as p4t, \
                 tc.tile_pool(name="pscan", bufs=1, space="PSUM") as pscan:
                for hb in range(4):
                    js = [hb * 2, hb * 2 + 1]
                    yps = {j: pscan.tile([P, L], f32, tag=f"yps{hb % 2}_{j % 2}",
                                         name=f"yps{j}") for j in js}
                    for n in range(D_STATE):
                        bcb = p4t.tile([P, 2 * L], fp16, tag="bcb",
                                       name=f"bcb{hb}_{n}", bufs=3)
                        nc.sync.dma_start(
                            bcb[:],
                            bcB2[n:n + 1, :].partition_broadcast(P).squeeze(1))
                        bcc = p4t.tile([P, 2 * L], fp16, tag="bcc",
                                       name=f"bcc{hb}_{n}", bufs=3)
                        nc.sync.dma_start(
                            bcc[:],
                            bcC2[n:n + 1, :].partition_broadcast(P).squeeze(1))
                        a_p = p4t.tile([P, 2 * L], fp16, tag="a_p",
                                       name=f"a{hb}_{n}", bufs=3)
                        for j in js:
                            nc.scalar.activation(
                                a_p[:, (j % 2) * L:(j % 2 + 1) * L],
                                deltas[j], Act.Exp,
                                scale=A_sb[:, j * D_STATE + n:j * D_STATE + n + 1])
                        b_p = p4t.tile([P, 2 * L], fp16, tag="b_p",
                                       name=f"b{hb}_{n}", bufs=2)
                        nc.vector.tensor_tensor(b_p[:], wpair[hb][:], bcb[:],
                                                Alu.mult)
                        h_p = p4t.tile([P, 2 * L], fp16, tag="h_p",
                                       name=f"h{hb}_{n}", bufs=2)
                        nc.vector.tensor_tensor_scan(h_p[:], a_p[:], b_p[:],
                                                     0.0, Alu.mult, Alu.add)
                        prod = p4t.tile([P, 2 * L], fp16, tag="prod",
                                        name=f"p{hb}_{n}", bufs=4)
                        nc.vector.tensor_tensor(prod[:], h_p[:], bcc[:], Alu.mult)
                        for j in js:
                            for t in range(NT):
                                sl = slice((j % 2) * L + t * TCH,
                                           (j % 2) * L + (t + 1) * TCH)
                                nc.tensor.matmul(
                                    yps[j][:, t * TCH:(t + 1) * TCH],
                                    ident_sb[:], prod[:, sl],
                                    start=(n == 0), stop=False)
                    for j in js:
                        for t in range(NT):
                            nc.tensor.matmul(
                                yps[j][:, t * TCH:(t + 1) * TCH],
                                Ddiag_sb[:, j * P:(j + 1) * P],
                                xcs[j][:, t * TCH:(t + 1) * TCH],
                                start=False, stop=True)
                    for j in js:
                        yc = p4t.tile([P, L], fp16, tag="yc", name=f"yc{j}",
                                      bufs=2)
                        nc.scalar.copy(yc[:], yps[j][:])
                        nc.vector.tensor_tensor(ygs[j][:], yc[:], szs[j][:],
                                                Alu.mult)

            # ================= P5: out_proj + RS =================
            with tc.tile_pool(name="p5t", bufs=1) as p5t, \
                 tc.tile_pool(name="psC", bufs=1, space="PSUM") as psC:
                dmy = p5t.tile([1, 4], f32, tag="dmy", name="dmy")
                nc.vector.memset(dmy[:], 0.5)
                for fi, fn in enumerate((Act.Square, Act.Sqrt, Act.Gelu)):
                    dmo = p5t.tile([1, 4], f32, tag="dmo", name=f"dmo{fi}", bufs=3)
                    nc.scalar.activation(dmo[:], dmy[:], fn)
                for m in range(NJ):
                    ms = p5t.tile([P, L], fp16, tag="ms", name=f"ms{m}", bufs=2)
                    for t in range(NT):
                        ps = psC.tile([P, TCH], f32, tag="ps", name=f"op{m}_{t}", bufs=2)
                        for k in range(NJ):
                            nc.tensor.matmul(ps[:],
                                             outw_sb[m][:, k * P:(k + 1) * P],
                                             ygs[k][:, t * TCH:(t + 1) * TCH],
                                             start=(k == 0), stop=(k == NJ - 1))
                        nc.scalar.copy(ms[:, t * TCH:(t + 1) * TCH], ps[:])
                    msF = p5t.tile([P, L], fp16, tag="msF", name=f"msF{m}", bufs=2)
                    nc.scalar.activation(msF[:], ms[:], Act.Copy,
                                         scale=flags_sb[:, 0:1])
                    msB = p5t.tile([P, L], fp16, tag="msB", name=f"msB{m}", bufs=2)
                    nc.scalar.activation(msB[:], ms[:, ::-1], Act.Copy,
                                         scale=flags_sb[:, 1:2])
                    msb = p5t.tile([P, L], fp16, tag="msb", name=f"msb{m}", bufs=2)
                    nc.vector.tensor_tensor(msb[:], msF[:], msB[:], Alu.add)
                    dst = arqs[m // 4]
                    for q in range(4):
                        nc.sync.dma_start(
                            dst[q, (m % 4) * P:(m % 4 + 1) * P, :],
                            msb[:, q * LQ:(q + 1) * LQ])
                    if m % 4 == 3:
                        nc.gpsimd.collective_compute(
                            "ReduceScatter", Alu.add, replica_groups=_QUADS,
                            ins=[arqs[m // 4][:]], outs=[rs_outs[m // 4][:]])

                # ================= P6: merge + LN =================
                # prefetch first FFN weight tiles while the RS is in flight
                w1_pre = []
                for m in range(6):
                    lw = p5t.tile([P, KD * P], bf16, tag="w1", name=f"w1_{m}",
                                  bufs=6)
                    nc.sync.dma_start(lw[:], w1m[m])
                    w1_pre.append(lw)
                w2_pre = p5t.tile([P, NH1 * P], bf16, tag="w2", name="w2_0",
                                  bufs=2)
                nc.sync.dma_start(w2_pre[:], w2m[0])

                mos = [p5t.tile([P, LQ], fp16, tag=f"mo{j}", name=f"mo{j}")
                       for j in range(KD)]
                mu_ps = psC.tile([1, LQ], f32, tag="mu", name="mu_ps")
                e2_ps = psC.tile([1, LQ], f32, tag="e2", name="e2_ps")
                for j in range(KD):
                    nc.sync.dma_start(mos[j][:],
                                      rs_outs[j // 4][(j % 4) * P:(j % 4 + 1) * P, :])
                    sq = p5t.tile([P, LQ], fp16, tag="sq", name=f"sq{j}", bufs=2)
                    nc.scalar.activation(sq[:], mos[j][:], Act.Square)
                    nc.tensor.matmul(mu_ps[:], ones_sb[:, 0:1], mos[j][:],
                                     start=(j == 0), stop=(j == KD - 1))
                    nc.tensor.matmul(e2_ps[:], ones_sb[:, 0:1], sq[:],
                                     start=(j == 0), stop=(j == KD - 1))
                mean = p5t.tile([1, LQ], f32, tag="mean", name="mean")
                nc.scalar.activation(mean[:], mu_ps[:], Act.Copy,
                                     scale=1.0 / D_MODEL)
                e2m = p5t.tile([1, LQ], f32, tag="e2m", name="e2m")
                nc.scalar.activation(e2m[:], e2_ps[:], Act.Copy,
                                     scale=1.0 / D_MODEL)
                m2 = p5t.tile([1, LQ], f32, tag="m2", name="m2")
                nc.vector.tensor_tensor(m2[:], mean[:], mean[:], Alu.mult)
                var = p5t.tile([1, LQ], f32, tag="var", name="var")
                nc.vector.tensor_tensor(var[:], e2m[:], m2[:], Alu.subtract)
                eps_sb = p5t.tile([1, 1], f32, tag="eps", name="eps_sb")
                nc.vector.memset(eps_sb[:], 1e-5)
                std = p5t.tile([1, LQ], f32, tag="std", name="std")
                nc.scalar.activation(std[:], var[:], Act.Sqrt, bias=eps_sb[:])
                rstd = p5t.tile([1, LQ], f32, tag="rstd", name="rstd")
                nc.vector.reciprocal(rstd[:], std[:])
                mean_h = p5t.tile([1, LQ], fp16, tag="mean_h", name="mean_h")
                nc.scalar.copy(mean_h[:], mean[:])
                rstd_h = p5t.tile([1, LQ], fp16, tag="rstd_h", name="rstd_h")
                nc.scalar.copy(rstd_h[:], rstd[:])
                mean_bc = psC.tile([P, LQ], f32, tag="mbc", name="mean_bc")
                nc.tensor.matmul(mean_bc[:], onesrow_sb[:], mean_h[:],
                                 start=True, stop=True)
                rstd_bc = psC.tile([P, LQ], f32, tag="rbc", name="rstd_bc")
                nc.tensor.matmul(rstd_bc[:], onesrow_sb[:], rstd_h[:],
                                 start=True, stop=True)

                xns = [p5t.tile([P, LQ], bf16, tag=f"xn{j}", name=f"xn{j}")
                       for j in range(KD)]
                for j in range(KD):
                    t1 = p5t.tile([P, LQ], f32, tag="lnt", name=f"lnt{j}", bufs=2)
                    nc.vector.tensor_tensor(t1[:], mos[j][:], mean_bc[:],
                                            Alu.subtract)
                    nc.vector.tensor_tensor(t1[:], t1[:], rstd_bc[:], Alu.mult)
                    nc.vector.tensor_scalar(xns[j][:], t1[:], lng_sb[:, j:j + 1],
                                            lnb_sb[:, j:j + 1], Alu.mult, Alu.add)

                # ================= P7: FFN =================
                with tc.tile_pool(name="ffh_pool", bufs=1) as ffh_pool:
                    ffhs = [ffh_pool.tile([P, LQ], bf16, tag=f"fh{m}",
                                          name=f"fh{m}") for m in range(NH1)]
                    for m in range(NH1):
                        if m < 6:
                            lw = w1_pre[m]
                        else:
                            lw = p5t.tile([P, KD * P], bf16, tag="w1",
                                          name=f"w1_{m}", bufs=6)
                            nc.sync.dma_start(lw[:], w1m[m])
                        ps = psC.tile([P, LQ], f32, tag="psf", name=f"f1{m}",
                                      bufs=2)
                        for k in range(KD):
                            nc.tensor.matmul(ps[:], lw[:, k * P:(k + 1) * P],
                                             xns[k][:],
                                             start=(k == 0), stop=(k == KD - 1))
                        nc.scalar.activation(ffhs[m][:], ps[:], Act.Gelu,
                                             bias=b1_sb[:, m:m + 1])

                    for m in range(KD):
                        if m == 0:
                            lw = w2_pre
                        else:
                            lw = p5t.tile([P, NH1 * P], bf16, tag="w2",
                                          name=f"w2_{m}", bufs=2)
                            nc.sync.dma_start(lw[:], w2m[m])
                        ps = psC.tile([P, LQ], f32, tag="psf", name=f"f2{m}",
                                      bufs=2)
                        for k in range(NH1):
                            nc.tensor.matmul(ps[:], lw[:, k * P:(k + 1) * P],
                                             ffhs[k][:],
                                             start=(k == 0), stop=(k == NH1 - 1))
                        ob = p5t.tile([P, LQ], f32, tag="ob", name=f"ob{m}",
                                      bufs=2)
                        nc.vector.tensor_scalar_add(ob[:], ps[:],
                                                    b2_sb[:, m:m + 1])
                        nc.sync.dma_start(out_m[m * P:(m + 1) * P, :], ob[:])

    nc.compile()
    return nc


def _prep_inputs(inputs):
    """Per-core input dicts. Core c: sequence s=c//2 (s>=2 => time-flipped x),
    d_inner half = c%2."""
    import ml_dtypes
    bf = ml_dtypes.bfloat16
    fh = np.float16

    x = np.asarray(inputs["x"], dtype=np.float32)
    in_proj_w = np.asarray(inputs["in_proj_w"], dtype=np.float32)
    conv_w = np.asarray(inputs["conv_w"], dtype=np.float32)
    conv_b = np.asarray(inputs["conv_b"], dtype=np.float32)
    x_proj_w = np.asarray(inputs["x_proj_w"], dtype=np.float32)
    dt_proj_w = np.asarray(inputs["dt_proj_w"], dtype=np.float32)
    dt_proj_b = np.asarray(inputs["dt_proj_b"], dtype=np.float32)
    A = -np.exp(np.asarray(inputs["A_log"], dtype=np.float32))
    Dp = np.asarray(inputs["D"], dtype=np.float32)
    out_proj_w = np.asarray(inputs["out_proj_w"], dtype=np.float32)
    ln_g = np.asarray(inputs["ln_g"], dtype=np.float32)
    ln_b = np.asarray(inputs["ln_b"], dtype=np.float32)
    ff_w1 = np.asarray(inputs["ff_w1"], dtype=np.float32)
    ff_b1 = np.asarray(inputs["ff_b1"], dtype=np.float32)
    ff_w2 = np.asarray(inputs["ff_w2"], dtype=np.float32)
    ff_b2 = np.asarray(inputs["ff_b2"], dtype=np.float32)

    def cols(v):  # (N,) -> (P, N//P)
        return np.ascontiguousarray(v.reshape(-1, P).T)

    def wblocks(w, dt):  # (K, M) -> (M//P, P(kpart), K//P, P(m))
        K, M = w.shape
        r = w.reshape(K // P, P, M // P, P).transpose(2, 1, 0, 3)
        return np.ascontiguousarray(r.astype(dt))

    # shared across cores
    w1_t = wblocks(ff_w1, bf)                      # (32, P, 8, P)
    w2_t = wblocks(ff_w2, bf)                      # (8, P, 32, P)
    lngc = cols(ln_g)
    lnbc = cols(ln_b)
    b1c = cols(ff_b1)
    b2c = cols(ff_b2)
    identh = np.eye(P, dtype=fh)
    ones_h = np.ones((P, 2), fh)
    onesrow_h = np.ones((1, P), fh)

    in_maps = []
    for c in range(8):
        s, half = c // 2, c % 2
        xb = x[s] if s < 2 else x[s - 2][::-1]
        own = np.arange(half * HALF, (half + 1) * HALF)

        wxc = in_proj_w[:, own]                    # (1024, 1024)
        wz = in_proj_w[:, D_INNER + own]
        w_in = np.concatenate(
            [wblocks(wxc, bf), wblocks(wz, bf)], axis=0)   # (16, P, 8, P)

        cw = conv_w[own]
        convw_cols = np.ascontiguousarray(
            cw.reshape(NJ, P, D_CONV).transpose(1, 0, 2).reshape(P, NJ * D_CONV))

        xp = np.concatenate(
            [x_proj_w[own], np.zeros((HALF, P - DT_RANK - 2 * D_STATE),
                                     np.float32)], axis=1)  # (1024, 128)
        xpw_t = wblocks(xp, fh)[0]                 # (P, 8, P)

        dtw_t = np.ascontiguousarray(
            dt_proj_w[:, own].reshape(DT_RANK, NJ, P).astype(fh))

        A_colsv = np.ascontiguousarray(
            A[own].reshape(NJ, P, D_STATE).transpose(1, 0, 2).reshape(
                P, NJ * D_STATE))

        outw_t = wblocks(out_proj_w[own], fh)      # (8, P, 8, P)

        Ddg = np.zeros((NJ, P, P), np.float32)
        for j in range(NJ):
            np.fill_diagonal(Ddg[j], Dp[own][j * P:(j + 1) * P])
        Ddg = np.ascontiguousarray(Ddg.transpose(1, 0, 2))
        fwd = 1.0 if s < 2 else 0.0
        flags = np.concatenate([np.full((P, 1), fwd, np.float32),
                                np.full((P, 1), 1.0 - fwd, np.float32)], axis=1)

        in_maps.append({
            "xT": np.ascontiguousarray(xb.T).astype(bf),
            "w_in": w_in,
            "convw_cols": convw_cols,
            "convb_cols": cols(conv_b[own]),
            "xpw": xpw_t,
            "dtw": dtw_t,
            "dtb_cols": cols(dt_proj_b[own]),
            "A_cols": A_colsv,
            "D_cols": cols(Dp[own]),
            "outw": outw_t,
            "lng_cols": lngc,
            "lnb_cols": lnbc,
            "w1m": w1_t,
            "b1_cols": b1c,
            "w2m": w2_t,
            "b2_cols": b2c,
            "identh": identh,
            "Ddiag": Ddg.astype(fh),
            "ones_h": ones_h,
            "onesrow_h": onesrow_h,
            "flags": flags,
        })
    return in_maps


_NC_CACHE = {}


def _get_nc():
    if "nc" not in _NC_CACHE:
        _NC_CACHE["nc"] = _build_nc()
    return _NC_CACHE["nc"]


def run(inputs, trace=False):
    _install_ntff_hook_shim()
    from concourse import bass_utils
    nc = _get_nc()
    in_maps = _prep_inputs(inputs)
    res = bass_utils.run_bass_kernel_spmd(nc, in_maps, core_ids=list(range(8)),
                                          trace=trace)
    # core at quad-rank q holds t-columns [q*256, (q+1)*256) of its batch
    full = np.zeros((2, D_MODEL, L), np.float32)
    for c in range(8):
        b = 0 if c in _QUADS[0] else 1
        q = _QUADS[b].index(c)
        full[b, :, q * LQ:(q + 1) * LQ] = res.results[c]["out_m"]
    out = np.ascontiguousarray(full.transpose(0, 2, 1))
    return out, res


def kernel(**inputs):
    out, _ = run(inputs, trace=False)
    return out


# revision 17
# speedup vs baseline: 1.0755x; 1.0217x over previous
"""BiMamba (bidirectional Mamba block + LN + FFN) Trainium2 Bass kernel.

Sharding (8 cores): 4 scan-sequences (fwd/bwd x batch, bwd fed host-flipped x)
x 2 halves of d_inner. Feature-on-partitions / time-on-free throughout.

Redesign vs baseline:
- Each core computes in_proj only for its own d_inner half (xc own + z own);
  the x_proj contraction over the full d_inner is completed with a pair
  AllReduce of the (128, L) x_proj partial sums.
- All large GEMMs run in bf16/fp16 (1 cycle/row + fast FWL weight loads).
- Scan phase all fp16: exp on Act engine, b/prod multiplies on DVE at the
  2x packed rate, tensor_tensor_scan fp16, state-sum via fp16 identity
  matmuls into PSUM.
- Direction merge + FFN input distribution via ONE ReduceScatter over quads
  that scatters along TIME: each core then owns a 256-column t-slice, does
  LN + the full FFN locally (weights streamed JIT), no further collectives.
  The bwd-core time flip is handled with per-core 0/1 flag columns scaling
  a straight and a reversed copy into separate RS slots (identical program
  on all cores).
"""
import sys, os, types, contextlib, ctypes

sys.path.insert(0, "/opt/trn_rl_repo")
import numpy as np

D_MODEL = 1024
D_STATE = 16
D_CONV = 4
D_INNER = 2048
DT_RANK = 64
L = 1024
HALF = D_INNER // 2          # 1024 d_inner per core
P = 128
NJ = HALF // P               # 8 d-blocks per core half
TCH = 512                    # matmul t-chunk
NT = L // TCH
KD = D_MODEL // P            # 8 k-chunks over d_model
NH1 = 4 * D_MODEL // P       # 32 ffn hidden blocks
LQ = L // 4                  # 256 t-slice per core after RS

_PAIRS = [[0, 1], [2, 3], [4, 5], [6, 7]]
_QUADS = [[0, 1, 4, 5], [2, 3, 6, 7]]
GP_N = ()


def _install_ntff_hook_shim(so_path="/opt/axon/libaxon_pjrt.so"):
    if "antenv.axon_hooks" in sys.modules:
        return
    try:
        lib = ctypes.CDLL(so_path)
    except OSError:
        return
    if not hasattr(lib, "axon_start_nrt_profile"):
        return
    lib.axon_start_nrt_profile.argtypes = [ctypes.POINTER(ctypes.c_int64), ctypes.c_size_t]
    lib.axon_start_nrt_profile.restype = ctypes.c_int64
    lib.axon_stop_nrt_profile.argtypes = [ctypes.c_char_p]
    lib.axon_stop_nrt_profile.restype = ctypes.c_int64

    @contextlib.contextmanager
    def _hook(output_dir, device_ids):
        import jax
        jax.devices()
        if device_ids:
            ids = (ctypes.c_int64 * len(device_ids))(*device_ids)
            rc = lib.axon_start_nrt_profile(ids, len(device_ids))
        else:
            rc = lib.axon_start_nrt_profile(None, 0)
        if rc != 0:
            raise RuntimeError(f"axon_start_nrt_profile rc={rc}")
        try:
            yield
        finally:
            n = lib.axon_stop_nrt_profile(str(output_dir).encode())
            print(f"profile: {n} file(s) written to {output_dir}", file=sys.stderr)

    mod = types.ModuleType("antenv.axon_hooks")
    mod.get_axon_ntff_profile_hook = lambda: _hook
    mod.set_axon_ntff_profile_hook = lambda h: None
    sys.modules["antenv.axon_hooks"] = mod


def _build_nc():
    from concourse import bacc, tile, mybir

    f32 = mybir.dt.float32
    bf16 = mybir.dt.bfloat16
    fp16 = mybir.dt.float16
    Alu = mybir.AluOpType
    Act = mybir.ActivationFunctionType

    nc = bacc.Bacc("TRN2", target_bir_lowering=False, debug=False, num_devices=8)

    def din(name, shape, dt):
        return nc.dram_tensor(name, list(shape), dt, kind="ExternalInput").ap()

    xT = din("xT", (D_MODEL, L), bf16)
    w_in = din("w_in", (16, P, KD, P), bf16)        # fb 0..7 xc-own, 8..15 z-own
    convw_cols = din("convw_cols", (P, NJ * D_CONV), f32)
    convb_cols = din("convb_cols", (P, NJ), f32)
    xpw = din("xpw", (P, NJ, P), fp16)              # [p, k, n] n: dt64|B16|C16|pad
    dtw = din("dtw", (DT_RANK, NJ, P), fp16)
    dtb_cols = din("dtb_cols", (P, NJ), f32)
    A_cols = din("A_cols", (P, NJ * D_STATE), f32)
    D_cols = din("D_cols", (P, NJ), f32)
    outw = din("outw", (NJ, P, NJ, P), fp16)        # [m, p(k-part), k, mp]
    lng_cols = din("lng_cols", (P, KD), f32)
    lnb_cols = din("lnb_cols", (P, KD), f32)
    w1m = din("w1m", (NH1, P, KD, P), bf16)
    b1_cols = din("b1_cols", (P, NH1), f32)
    w2m = din("w2m", (KD, P, NH1, P), bf16)
    b2_cols = din("b2_cols", (P, KD), f32)
    identh = din("identh", (P, P), fp16)
    Ddiag = din("Ddiag", (P, NJ, P), fp16)
    ones_h = din("ones_h", (P, 2), fp16)            # col0: ones (stats lhsT)
    onesrow_h = din("onesrow_h", (1, P), fp16)      # bcast lhsT
    flags = din("flags", (P, 2), f32)               # col0 fwd, col1 bwd

    out_m = nc.dram_tensor("out_m", [D_MODEL, LQ], f32, kind="ExternalOutput").ap()

    with tile.TileContext(nc) as tc:
        with contextlib.ExitStack() as stk:
            cpool = stk.enter_context(tc.tile_pool(name="cpool", bufs=1))
            dram = stk.enter_context(tc.tile_pool(name="dram", bufs=1, space="DRAM"))

            def cload(src, shape, dt, tag):
                t = cpool.tile(list(shape), dt, tag=tag, name=tag)
                nc.sync.dma_start(t[:], src)
                return t

            # input activations first in the DMA queue
            xts = []
            for k in range(KD):
                xt_k = cpool.tile([P, L], bf16, tag=f"xt{k}", name=f"xt{k}")
                nc.sync.dma_start(xt_k[:], xT[k * P:(k + 1) * P, :])
                xts.append(xt_k)

            convw_sb = cload(convw_cols[:], (P, NJ * D_CONV), f32, "convw_sb")
            convb_sb = cload(convb_cols[:], (P, NJ), f32, "convb_sb")
            dtb_sb = cload(dtb_cols[:], (P, NJ), f32, "dtb_sb")
            A_sb = cload(A_cols[:], (P, NJ * D_STATE), f32, "A_sb")
            D_sb = cload(D_cols[:], (P, NJ), f32, "D_sb")
            lng_sb = cload(lng_cols[:], (P, KD), f32, "lng_sb")
            lnb_sb = cload(lnb_cols[:], (P, KD), f32, "lnb_sb")
            b1_sb = cload(b1_cols[:], (P, NH1), f32, "b1_sb")
            b2_sb = cload(b2_cols[:], (P, KD), f32, "b2_sb")
            ident_sb = cload(identh[:], (P, P), fp16, "ident_sb")
            Ddiag_sb = cload(Ddiag[:], (P, NJ * P), fp16, "Ddiag_sb")
            ones_sb = cload(ones_h[:], (P, 2), fp16, "ones_sb")
            onesrow_sb = cload(onesrow_h[:], (1, P), fp16, "onesrow_sb")
            flags_sb = cload(flags[:], (P, 2), f32, "flags_sb")
            xpw_sb = cload(xpw[:], (P, NJ * P), fp16, "xpw_sb")
            dtw_sb = cload(dtw[:], (DT_RANK, NJ * P), fp16, "dtw_sb")

            dbl_in = dram.tile([P, L], fp16, name="dbl_in")
            dbl_out = dram.tile([P, L], fp16, name="dbl_out")
            bcB2 = dram.tile([D_STATE, 2 * L], fp16, name="bcB2")
            bcC2 = dram.tile([D_STATE, 2 * L], fp16, name="bcC2")
            arqs = [dram.tile([4, D_MODEL // 2, LQ], fp16, name=f"arq{i}")
                    for i in range(2)]
            rs_outs = [dram.tile([D_MODEL // 2, LQ], fp16, name=f"rso{i}")
                       for i in range(2)]

            # persistent SBUF (P1->P4/P5)
            sz_pool = stk.enter_context(tc.tile_pool(name="sz_pool", bufs=1))
            szs = [sz_pool.tile([P, L], fp16, tag=f"sz{j}", name=f"sz{j}")
                   for j in range(NJ)]
            dl_pool = stk.enter_context(tc.tile_pool(name="dl_pool", bufs=1))
            dpair = [dl_pool.tile([P, 2 * L], fp16, tag=f"dl{g}", name=f"dl{g}")
                     for g in range(NJ // 2)]
            deltas = [dpair[j // 2][:, (j % 2) * L:(j % 2 + 1) * L]
                      for j in range(NJ)]
            wv_pool = stk.enter_context(tc.tile_pool(name="wv_pool", bufs=1))
            wpair = [wv_pool.tile([P, 2 * L], fp16, tag=f"wv{g}", name=f"wv{g}")
                     for g in range(NJ // 2)]
            wvs = [wpair[j // 2][:, (j % 2) * L:(j % 2 + 1) * L]
                   for j in range(NJ)]
            xc_pool = stk.enter_context(tc.tile_pool(name="xc_pool", bufs=1))
            xcpair = [xc_pool.tile([P, 2 * L], fp16, tag=f"xc{g}",
                                   name=f"xc{g}") for g in range(NJ // 2)]
            xcs = [xcpair[j // 2][:, (j % 2) * L:(j % 2 + 1) * L]
                   for j in range(NJ)]
            yg_pool = stk.enter_context(tc.tile_pool(name="yg_pool", bufs=1))
            ygs = [yg_pool.tile([P, L], fp16, tag=f"yg{j}", name=f"yg{j}")
                   for j in range(NJ)]

            # ================= P1-P3 =================
            with tc.tile_pool(name="p13", bufs=1) as p13, \
                 tc.tile_pool(name="psA", bufs=4, space="PSUM") as psA:

                def in_proj_block(fb, tag):
                    lw = p13.tile([P, KD * P], bf16, tag=tag, name=f"{tag}_{fb}",
                                  bufs=2)
                    nc.sync.dma_start(lw[:], w_in[fb])
                    pss = []
                    for t in range(NT):
                        ps = psA.tile([P, TCH], f32, tag="ps", name=f"inp{fb}_{t}")
                        for k in range(KD):
                            nc.tensor.matmul(ps[:], lw[:, k * P:(k + 1) * P],
                                             xts[k][:, t * TCH:(t + 1) * TCH],
                                             start=(k == 0), stop=(k == KD - 1))
                        pss.append(ps)
                    return pss

                # P1a: xc own half + conv + silu
                for j in range(NJ):
                    xcp = p13.tile([P, L + D_CONV - 1], fp16, tag="xcp",
                                   name=f"xcp{j}", bufs=2)
                    nc.vector.memset(xcp[:, 0:D_CONV - 1], 0.0)
                    for t, ps in enumerate(in_proj_block(j, "lwx")):
                        nc.scalar.copy(
                            xcp[:, D_CONV - 1 + t * TCH:D_CONV - 1 + (t + 1) * TCH],
                            ps[:])
                    cacc = p13.tile([P, L], fp16, tag="cacc", name=f"cacc{j}",
                                    bufs=2)
                    nc.vector.tensor_scalar_mul(
                        cacc[:], xcp[:, 0:L],
                        convw_sb[:, j * D_CONV:j * D_CONV + 1])
                    for i in range(1, D_CONV):
                        nc.vector.scalar_tensor_tensor(
                            cacc[:], xcp[:, i:i + L],
                            convw_sb[:, j * D_CONV + i:j * D_CONV + i + 1],
                            cacc[:], Alu.mult, Alu.add)
                    nc.scalar.activation(xcs[j], cacc[:], Act.Silu,
                                         bias=convb_sb[:, j:j + 1])

                # P1b: x_proj partial over own half + pair AllReduce
                dblp = p13.tile([P, L], fp16, tag="dblp", name="dblp")
                for t in range(NT):
                    ps = psA.tile([P, TCH], f32, tag="ps", name=f"xp{t}")
                    for k in range(NJ):
                        nc.tensor.matmul(ps[:], xpw_sb[:, k * P:(k + 1) * P],
                                         xcs[k][:, t * TCH:(t + 1) * TCH],
                                         start=(k == 0), stop=(k == NJ - 1))
                    nc.scalar.copy(dblp[:, t * TCH:(t + 1) * TCH], ps[:])
                nc.sync.dma_start(dbl_in[:], dblp[:])
                nc.gpsimd.collective_compute(
                    "AllReduce", Alu.add, replica_groups=_PAIRS,
                    ins=[dbl_in[:]], outs=[dbl_out[:]])

                # P1c: z own half + silu, first half (overlaps the AllReduce)
                for j in range(NJ // 2):
                    for t, ps in enumerate(in_proj_block(NJ + j, "lwz")):
                        nc.scalar.activation(szs[j][:, t * TCH:(t + 1) * TCH],
                                             ps[:], Act.Silu)

                # P2: unpack AllReduce result (fp16 throughout)
                dt16 = p13.tile([DT_RANK, L], fp16, tag="dt16", name="dt16")
                nc.sync.dma_start(dt16[:], dbl_out[0:DT_RANK, :])
                nc.sync.dma_start(bcB2[:, 0:L],
                                  dbl_out[DT_RANK:DT_RANK + D_STATE, :])
                nc.sync.dma_start(bcB2[:, L:2 * L],
                                  dbl_out[DT_RANK:DT_RANK + D_STATE, :])
                nc.sync.dma_start(bcC2[:, 0:L],
                                  dbl_out[DT_RANK + D_STATE:DT_RANK + 2 * D_STATE, :])
                nc.sync.dma_start(bcC2[:, L:2 * L],
                                  dbl_out[DT_RANK + D_STATE:DT_RANK + 2 * D_STATE, :])

                # P2b: dt_proj + softplus -> delta (fp16); batch Exp then Ln
                spts = {}
                for j in range(NJ):
                    for t in range(NT):
                        ps = psA.tile([P, TCH], f32, tag="ps", name=f"dtp{j}_{t}")
                        nc.tensor.matmul(ps[:], dtw_sb[:, j * P:(j + 1) * P],
                                         dt16[:, t * TCH:(t + 1) * TCH],
                                         start=True, stop=True)
                        spt = p13.tile([P, TCH], fp16, tag=f"sp{j}_{t}",
                                       name=f"spt{j}_{t}")
                        nc.scalar.activation(spt[:], ps[:], Act.Exp,
                                             bias=dtb_sb[:, j:j + 1])
                        spts[(j, t)] = spt
                for j in range(NJ):
                    for t in range(NT):
                        nc.scalar.activation(deltas[j][:, t * TCH:(t + 1) * TCH],
                                             spts[(j, t)][:], Act.Ln, bias=1.0)

                # P1c cont: z own half, second half (after dt_proj matmuls)
                for j in range(NJ // 2, NJ):
                    for t, ps in enumerate(in_proj_block(NJ + j, "lwz")):
                        nc.scalar.activation(szs[j][:, t * TCH:(t + 1) * TCH],
                                             ps[:], Act.Silu)

                # P3: wv (paired), then clobber pair-boundary delta
                for g in range(NJ // 2):
                    nc.vector.tensor_tensor(wpair[g][:], dpair[g][:],
                                            xcpair[g][:], Alu.mult)
                    nc.vector.memset(dpair[g][:, L:L + 1], 30.0)

            # out_proj weights (2 MB fp16) load during the scan phase
            outw_sb = [cload(outw[m], (P, NJ * P), fp16, f"outw{m}")
                       for m in range(NJ)]

            # ================= P4: scan =================
            with tc.tile_pool(name="p4t", bufs=1) as p4t, \
                 tc.tile_pool(name="pscan", bufs=1, space="PSUM") as pscan:
                for hb in range(4):
                    js = [hb * 2, hb * 2 + 1]
                    yps = {j: pscan.tile([P, L], f32, tag=f"yps{hb % 2}_{j % 2}",
                                         name=f"yps{j}") for j in js}
                    for n in range(D_STATE):
                        bcb = p4t.tile([P, 2 * L], fp16, tag="bcb",
                                       name=f"bcb{hb}_{n}", bufs=3)
                        nc.sync.dma_start(
                            bcb[:],
                            bcB2[n:n + 1, :].partition_broadcast(P).squeeze(1))
                        bcc = p4t.tile([P, 2 * L], fp16, tag="bcc",
                                       name=f"bcc{hb}_{n}", bufs=3)
                        nc.sync.dma_start(
                            bcc[:],
                            bcC2[n:n + 1, :].partition_broadcast(P).squeeze(1))
                        a_p = p4t.tile([P, 2 * L], fp16, tag="a_p",
                                       name=f"a{hb}_{n}", bufs=3)
                        for j in js:
                            nc.scalar.activation(
                                a_p[:, (j % 2) * L:(j % 2 + 1) * L],
                                deltas[j], Act.Exp,
                                scale=A_sb[:, j * D_STATE + n:j * D_STATE + n + 1])
                        b_p = p4t.tile([P, 2 * L], fp16, tag="b_p",
                                       name=f"b{hb}_{n}", bufs=2)
                        nc.vector.tensor_tensor(b_p[:], wpair[hb][:], bcb[:],
                                                Alu.mult)
                        h_p = p4t.tile([P, 2 * L], fp16, tag="h_p",
                                       name=f"h{hb}_{n}", bufs=2)
                        nc.vector.tensor_tensor_scan(h_p[:], a_p[:], b_p[:],
                                                     0.0, Alu.mult, Alu.add)
                        prod = p4t.tile([P, 2 * L], fp16, tag="prod",
                                        name=f"p{hb}_{n}", bufs=4)
                        nc.vector.tensor_tensor(prod[:], h_p[:], bcc[:], Alu.mult)
                        for j in js:
                            for t in range(NT):
                                sl = slice((j % 2) * L + t * TCH,
                                           (j % 2) * L + (t + 1) * TCH)
                                nc.tensor.matmul(
                                    yps[j][:, t * TCH:(t + 1) * TCH],
                                    ident_sb[:], prod[:, sl],
                                    start=(n == 0), stop=False)
                    for j in js:
                        for t in range(NT):
                            nc.tensor.matmul(
                                yps[j][:, t * TCH:(t + 1) * TCH],
                                Ddiag_sb[:, j * P:(j + 1) * P],
                                xcs[j][:, t * TCH:(t + 1) * TCH],
                                start=False, stop=True)
                    for j in js:
                        yc = p4t.tile([P, L], fp16, tag="yc", name=f"yc{j}",
                                      bufs=2)
                        nc.scalar.copy(yc[:], yps[j][:])
                        nc.vector.tensor_tensor(ygs[j][:], yc[:], szs[j][:],
                                                Alu.mult)

            # ================= P5: out_proj + RS =================
            with tc.tile_pool(name="p5t", bufs=1) as p5t, \
                 tc.tile_pool(name="psC", bufs=1, space="PSUM") as psC:
                for m in range(NJ):
                    ms = p5t.tile([P, L], fp16, tag="ms", name=f"ms{m}", bufs=2)
                    for t in range(NT):
                        ps = psC.tile([P, TCH], f32, tag="ps", name=f"op{m}_{t}", bufs=2)
                        for k in range(NJ):
                            nc.tensor.matmul(ps[:],
                                             outw_sb[m][:, k * P:(k + 1) * P],
                                             ygs[k][:, t * TCH:(t + 1) * TCH],
                                             start=(k == 0), stop=(k == NJ - 1))
                        nc.scalar.copy(ms[:, t * TCH:(t + 1) * TCH], ps[:])
                    msF = p5t.tile([P, L], fp16, tag="msF", name=f"msF{m}", bufs=2)
                    nc.scalar.activation(msF[:], ms[:], Act.Copy,
                                         scale=flags_sb[:, 0:1])
                    msB = p5t.tile([P, L], fp16, tag="msB", name=f"msB{m}", bufs=2)
                    nc.scalar.activation(msB[:], ms[:, ::-1], Act.Copy,
                                         scale=flags_sb[:, 1:2])
                    msb = p5t.tile([P, L], fp16, tag="msb", name=f"msb{m}", bufs=2)
                    nc.vector.tensor_tensor(msb[:], msF[:], msB[:], Alu.add)
                    dst = arqs[m // 4]
                    for q in range(4):
                        nc.sync.dma_start(
                            dst[q, (m % 4) * P:(m % 4 + 1) * P, :],
                            msb[:, q * LQ:(q + 1) * LQ])
                    if m % 4 == 3:
                        nc.gpsimd.collective_compute(
                            "ReduceScatter", Alu.add, replica_groups=_QUADS,
                            ins=[arqs[m // 4][:]], outs=[rs_outs[m // 4][:]])

                # ================= P6: merge + LN =================
                # prefetch first FFN weight tiles while the RS is in flight
                w1_pre = []
                for m in range(6):
                    lw = p5t.tile([P, KD * P], bf16, tag="w1", name=f"w1_{m}",
                                  bufs=6)
                    nc.sync.dma_start(lw[:], w1m[m])
                    w1_pre.append(lw)
                w2_pre = p5t.tile([P, NH1 * P], bf16, tag="w2", name="w2_0",
                                  bufs=2)
                nc.sync.dma_start(w2_pre[:], w2m[0])

                mos = [p5t.tile([P, LQ], fp16, tag=f"mo{j}", name=f"mo{j}")
                       for j in range(KD)]
                mu_ps = psC.tile([1, LQ], f32, tag="mu", name="mu_ps")
                e2_ps = psC.tile([1, LQ], f32, tag="e2", name="e2_ps")
                for j in range(KD):
                    nc.sync.dma_start(mos[j][:],
                                      rs_outs[j // 4][(j % 4) * P:(j % 4 + 1) * P, :])
                    sq = p5t.tile([P, LQ], fp16, tag="sq", name=f"sq{j}", bufs=2)
                    nc.scalar.activation(sq[:], mos[j][:], Act.Square)
                    nc.tensor.matmul(mu_ps[:], ones_sb[:, 0:1], mos[j][:],
                                     start=(j == 0), stop=(j == KD - 1))
                    nc.tensor.matmul(e2_ps[:], ones_sb[:, 0:1], sq[:],
                                     start=(j == 0), stop=(j == KD - 1))
                mean = p5t.tile([1, LQ], f32, tag="mean", name="mean")
                nc.scalar.activation(mean[:], mu_ps[:], Act.Copy,
                                     scale=1.0 / D_MODEL)
                e2m = p5t.tile([1, LQ], f32, tag="e2m", name="e2m")
                nc.scalar.activation(e2m[:], e2_ps[:], Act.Copy,
                                     scale=1.0 / D_MODEL)
                m2 = p5t.tile([1, LQ], f32, tag="m2", name="m2")
                nc.vector.tensor_tensor(m2[:], mean[:], mean[:], Alu.mult)
                var = p5t.tile([1, LQ], f32, tag="var", name="var")
                nc.vector.tensor_tensor(var[:], e2m[:], m2[:], Alu.subtract)
                eps_sb = p5t.tile([1, 1], f32, tag="eps", name="eps_sb")
                nc.vector.memset(eps_sb[:], 1e-5)
                std = p5t.tile([1, LQ], f32, tag="std", name="std")
                nc.scalar.activation(std[:], var[:], Act.Sqrt, bias=eps_sb[:])
                rstd = p5t.tile([1, LQ], f32, tag="rstd", name="rstd")
                nc.vector.reciprocal(rstd[:], std[:])
                mean_h = p5t.tile([1, LQ], fp16, tag="mean_h", name="mean_h")
                nc.scalar.copy(mean_h[:], mean[:])
                rstd_h = p5t.tile([1, LQ], fp16, tag="rstd_h", name="rstd_h")
                nc.scalar.copy(rstd_h[:], rstd[:])
                mean_bc = psC.tile([P, LQ], f32, tag="mbc", name="mean_bc")
                nc.tensor.matmul(mean_bc[:], onesrow_sb[:], mean_h[:],
                                 start=True, stop=True)
                rstd_bc = psC.tile([P, LQ], f32, tag="rbc", name="rstd_bc")
                nc.tensor.matmul(rstd_bc[:], onesrow_sb[:], rstd_h[:],
                                 start=True, stop=True)

                xns = [p5t.tile([P, LQ], bf16, tag=f"xn{j}", name=f"xn{j}")
                       for j in range(KD)]
                for j in range(KD):
                    t1 = p5t.tile([P, LQ], f32, tag="lnt", name=f"lnt{j}", bufs=2)
                    nc.vector.tensor_tensor(t1[:], mos[j][:], mean_bc[:],
                                            Alu.subtract)
                    nc.vector.tensor_tensor(t1[:], t1[:], rstd_bc[:], Alu.mult)
                    nc.vector.tensor_scalar(xns[j][:], t1[:], lng_sb[:, j:j + 1],
                                            lnb_sb[:, j:j + 1], Alu.mult, Alu.add)

                # ================= P7: FFN =================
                with tc.tile_pool(name="ffh_pool", bufs=1) as ffh_pool:
                    ffhs = [ffh_pool.tile([P, LQ], bf16, tag=f"fh{m}",
                                          name=f"fh{m}") for m in range(NH1)]
                    for m in range(NH1):
                        if m < 6:
                            lw = w1_pre[m]
                        else:
                            lw = p5t.tile([P, KD * P], bf16, tag="w1",
                                          name=f"w1_{m}", bufs=6)
                            nc.sync.dma_start(lw[:], w1m[m])
                        ps = psC.tile([P, LQ], f32, tag="psf", name=f"f1{m}",
                                      bufs=2)
                        for k in range(KD):
                            nc.tensor.matmul(ps[:], lw[:, k * P:(k + 1) * P],
                                             xns[k][:],
                                             start=(k == 0), stop=(k == KD - 1))
                        nc.scalar.activation(ffhs[m][:], ps[:], Act.Gelu,
                                             bias=b1_sb[:, m:m + 1])

                    for m in range(KD):
                        if m == 0:
                            lw = w2_pre
                        else:
                            lw = p5t.tile([P, NH1 * P], bf16, tag="w2",
                                          name=f"w2_{m}", bufs=2)
                            nc.sync.dma_start(lw[:], w2m[m])
                        ps = psC.tile([P, LQ], f32, tag="psf", name=f"f2{m}",
                                      bufs=2)
                        for k in range(NH1):
                            nc.tensor.matmul(ps[:], lw[:, k * P:(k + 1) * P],
                                             ffhs[k][:],
                                             start=(k == 0), stop=(k == NH1 - 1))
                        ob = p5t.tile([P, LQ], f32, tag="ob", name=f"ob{m}",
                                      bufs=2)
                        nc.vector.tensor_scalar_add(ob[:], ps[:],
                                                    b2_sb[:, m:m + 1])
                        nc.sync.dma_start(out_m[m * P:(m + 1) * P, :], ob[:])

    nc.compile()
    return nc


def _prep_inputs(inputs):
    """Per-core input dicts. Core c: sequence s=c//2 (s>=2 => time-flipped x),
    d_inner half = c%2."""
    import ml_dtypes
    bf = ml_dtypes.bfloat16
    fh = np.float16

    x = np.asarray(inputs["x"], dtype=np.float32)
    in_proj_w = np.asarray(inputs["in_proj_w"], dtype=np.float32)
    conv_w = np.asarray(inputs["conv_w"], dtype=np.float32)
    conv_b = np.asarray(inputs["conv_b"], dtype=np.float32)
    x_proj_w = np.asarray(inputs["x_proj_w"], dtype=np.float32)
    dt_proj_w = np.asarray(inputs["dt_proj_w"], dtype=np.float32)
    dt_proj_b = np.asarray(inputs["dt_proj_b"], dtype=np.float32)
    A = -np.exp(np.asarray(inputs["A_log"], dtype=np.float32))
    Dp = np.asarray(inputs["D"], dtype=np.float32)
    out_proj_w = np.asarray(inputs["out_proj_w"], dtype=np.float32)
    ln_g = np.asarray(inputs["ln_g"], dtype=np.float32)
    ln_b = np.asarray(inputs["ln_b"], dtype=np.float32)
    ff_w1 = np.asarray(inputs["ff_w1"], dtype=np.float32)
    ff_b1 = np.asarray(inputs["ff_b1"], dtype=np.float32)
    ff_w2 = np.asarray(inputs["ff_w2"], dtype=np.float32)
    ff_b2 = np.asarray(inputs["ff_b2"], dtype=np.float32)

    def cols(v):  # (N,) -> (P, N//P)
        return np.ascontiguousarray(v.reshape(-1, P).T)

    def wblocks(w, dt):  # (K, M) -> (M//P, P(kpart), K//P, P(m))
        K, M = w.shape
        r = w.reshape(K // P, P, M // P, P).transpose(2, 1, 0, 3)
        return np.ascontiguousarray(r.astype(dt))

    # shared across cores
    w1_t = wblocks(ff_w1, bf)                      # (32, P, 8, P)
    w2_t = wblocks(ff_w2, bf)                      # (8, P, 32, P)
    lngc = cols(ln_g)
    lnbc = cols(ln_b)
    b1c = cols(ff_b1)
    b2c = cols(ff_b2)
    identh = np.eye(P, dtype=fh)
    ones_h = np.ones((P, 2), fh)
    onesrow_h = np.ones((1, P), fh)

    in_maps = []
    for c in range(8):
        s, half = c // 2, c % 2
        xb = x[s] if s < 2 else x[s - 2][::-1]
        own = np.arange(half * HALF, (half + 1) * HALF)

        wxc = in_proj_w[:, own]                    # (1024, 1024)
        wz = in_proj_w[:, D_INNER + own]
        w_in = np.concatenate(
            [wblocks(wxc, bf), wblocks(wz, bf)], axis=0)   # (16, P, 8, P)

        cw = conv_w[own]
        convw_cols = np.ascontiguousarray(
            cw.reshape(NJ, P, D_CONV).transpose(1, 0, 2).reshape(P, NJ * D_CONV))

        xp = np.concatenate(
            [x_proj_w[own], np.zeros((HALF, P - DT_RANK - 2 * D_STATE),
                                     np.float32)], axis=1)  # (1024, 128)
        xpw_t = wblocks(xp, fh)[0]                 # (P, 8, P)

        dtw_t = np.ascontiguousarray(
            dt_proj_w[:, own].reshape(DT_RANK, NJ, P).astype(fh))

        A_colsv = np.ascontiguousarray(
            A[own].reshape(NJ, P, D_STATE).transpose(1, 0, 2).reshape(
                P, NJ * D_STATE))

        outw_t = wblocks(out_proj_w[own], fh)      # (8, P, 8, P)

        Ddg = np.zeros((NJ, P, P), np.float32)
        for j in range(NJ):
            np.fill_diagonal(Ddg[j], Dp[own][j * P:(j + 1) * P])
        Ddg = np.ascontiguousarray(Ddg.transpose(1, 0, 2))
        fwd = 1.0 if s < 2 else 0.0
        flags = np.concatenate([np.full((P, 1), fwd, np.float32),
                                np.full((P, 1), 1.0 - fwd, np.float32)], axis=1)

        in_maps.append({
            "xT": np.ascontiguousarray(xb.T).astype(bf),
            "w_in": w_in,
            "convw_cols": convw_cols,
            "convb_cols": cols(conv_b[own]),
            "xpw": xpw_t,
            "dtw": dtw_t,
            "dtb_cols": cols(dt_proj_b[own]),
            "A_cols": A_colsv,
            "D_cols": cols(Dp[own]),
            "outw": outw_t,
            "lng_cols": lngc,
            "lnb_cols": lnbc,
            "w1m": w1_t,
            "b1_cols": b1c,
            "w2m": w2_t,
            "b2_cols": b2c,
            "identh": identh,
            "Ddiag": Ddg.astype(fh),
            "ones_h": ones_h,
            "onesrow_h": onesrow_h,
            "flags": flags,
        })
    return in_maps


_NC_CACHE = {}


def _get_nc():
    if "nc" not in _NC_CACHE:
        _NC_CACHE["nc"] = _build_nc()
    return _NC_CACHE["nc"]


def run(inputs, trace=False):
    _install_ntff_hook_shim()
    from concourse import bass_utils
    nc = _get_nc()
    in_maps = _prep_inputs(inputs)
    res = bass_utils.run_bass_kernel_spmd(nc, in_maps, core_ids=list(range(8)),
                                          trace=trace)
    # core at quad-rank q holds t-columns [q*256, (q+1)*256) of its batch
    full = np.zeros((2, D_MODEL, L), np.float32)
    for c in range(8):
        b = 0 if c in _QUADS[0] else 1
        q = _QUADS[b].index(c)
        full[b, :, q * LQ:(q + 1) * LQ] = res.results[c]["out_m"]
    out = np.ascontiguousarray(full.transpose(0, 2, 1))
    return out, res


def kernel(**inputs):
    out, _ = run(inputs, trace=False)
    return out
